# revision 25
# baseline (speedup 1.0000x reference)
"""Trainium2 Bass kernel for gated relative-position attention.

Problem (hardcoded shapes): B=8, S=1024, E=512, H=8 heads, D=64.
    q = query @ Wq.T ; k = key @ Wk.T ; v = value @ Wv.T      (per-head split)
    content = softmax(q k^T / sqrt(E)) ; pos = softmax(mask)
    attn = (1-sigmoid(g)) * content + sigmoid(g) * pos ; attn /= attn.sum(-1)
    out  = attn @ v
Returns (out [B,S,E], attn [B,H,S,S]).

Sharding: data parallel over batch — core c computes batch element c
(all 8 heads). No collectives; host scatters inputs / gathers outputs.

Per-core pipeline (all fp32):
  prologue: PE-transpose query/key/value/W to get X^T layouts, project
            q^T,k^T (head-dim on partitions) and v (natural), sigmoid gates.
  per (head, q-block of 128 rows):
    PE:  scores[128,1024] = q^T.T @ k^T           (K=D=64)
    ACT: cexp = Exp(scale*scores)  with accum_out -> row sums (free)
    ACT: pexp = Exp(mask tile)     with accum_out -> row sums
    DVE: a_row=(1-g)/csum, b_row=g/psum  (per-partition scalars)
    DVE: attn = (cexp*a_row) + (pexp*b_row)   [tensor_scalar + scalar_tensor_tensor]
    PE:  transpose attn 128x128 blocks -> attn_T ; ACT/DVE copy PSUM->SBUF
  per head: PE: out^T[64,1024] = sum_k v_k^T-block matmuls over attn_T
  (renormalization by attn.sum(-1) is skipped: both softmaxes sum to 1,
   so the sum is 1 +- ~1e-6 and the division is a no-op at fp32 scale)

Host gathers attn directly and transposes out^T -> out (numpy).
"""

import os

os.environ.setdefault("MYCRO_LOCAL_CACHE", "1")

import numpy as np

import concourse.bass as bass
import concourse.mybir as mybir
import concourse.tile as tile
from concourse import bacc
from concourse.bass_utils import run_bass_kernel_spmd
from concourse.masks import make_identity

S = 1024
E = 512
H = 8
D = 64
NCORES = 8
SCALE = float(E) ** -0.5
F32 = mybir.dt.float32
F32R = mybir.dt.float32r
BF16 = mybir.dt.bfloat16
USE_F32R = os.environ.get("KERNEL_F32R", "1") == "1"
MM_DT = F32R if USE_F32R else F32
AF = mybir.ActivationFunctionType
ALU = mybir.AluOpType

SB = S // 128  # 8 s-blocks
EB = E // 128  # 4 e-blocks

LAST_RESULTS = None  # BassKernelResults of the most recent run (for test.py)


def _emit(nc, tc, ctx, tensors):
    query, key, value, Wq, Wk, Wv, gating, mask, attn_out, out_t = tensors

    pers = ctx.enter_context(tc.tile_pool(name="pers", bufs=1))

    # --- constants -------------------------------------------------------
    identity = pers.tile([128, 128], F32)
    make_identity(nc, identity)
    identity_r = pers.tile([128, 128], MM_DT)
    nc.vector.tensor_copy(identity_r, identity)
    ones1 = pers.tile([1, 128], F32)
    nc.vector.memset(ones1, 1.0)

    # --- gates: sigmoid via exp + reciprocal (stays in exp table set) ----
    # broadcast gating to all 128 partitions via 0-stride DMA, then compute
    # sigmoid replicated per partition (no PSUM / matmul involved).
    g_in = pers.tile([128, H], F32)
    nc.sync.dma_start(out=g_in, in_=gating.ap().partition_broadcast(128))
    g_eneg = pers.tile([128, H], F32)
    nc.scalar.activation(g_eneg, g_in, AF.Exp, scale=-1.0)
    g_den = pers.tile([128, H], F32)
    nc.vector.tensor_scalar(g_den, g_eneg, 1.0, None, ALU.add)
    gbc = pers.tile([128, 3 * H], F32)  # cols 0..7 = g, 8..15 = 1-g, 16..23 = g/(1-g)
    nc.vector.reciprocal(gbc[:, 0:H], g_den)
    nc.vector.tensor_scalar(gbc[:, H : 2 * H], gbc[:, 0:H], -1.0, 1.0, ALU.mult, ALU.add)
    grat = pers.tile([128, H], F32)
    nc.vector.reciprocal(grat, gbc[:, H : 2 * H])
    nc.vector.tensor_tensor(gbc[:, 2 * H : 3 * H], gbc[:, 0:H], grat, ALU.mult)

    # --- prologue: transposes + projections ------------------------------
    qT_all = pers.tile([128, EB, S], BF16)  # (q W^T)^T: head-dim on partitions
    kT_all = pers.tile([128, EB, S], BF16)
    vp = pers.tile([128, SB, E], MM_DT)      # v natural: [s-block partitions, h*64+d]


    mask_ap = mask.ap()

    with (
        tc.tile_pool(name="pro", bufs=2) as pro,
        tc.tile_pool(name="proT", bufs=1) as proT,
        tc.tile_pool(name="propsum", bufs=2, space="PSUM") as ppsum,
    ):
        # sacrificial transpose: absorbs the gpsimd(identity) wait on PE so
        # later PE instructions don't each need a Pool semaphore wait.
        warm = ppsum.tile([128, S], F32, name="warm", tag="pps")
        nc.tensor.transpose(warm[:, 0:128], identity, identity)

        xT_tiles = {}
        for name, dram in (("query", query), ("key", key), ("value", value)):
            xdt = MM_DT if name == "value" else BF16
            xT = proT.tile([128, EB, S], xdt, name=f"{name}T")
            # one DMA: [s, e] -> [s%128, s//128, e]
            xn = pro.tile([128, SB, E], F32, name=f"{name}_nat", tag="xnat")
            nc.sync.dma_start(
                out=xn, in_=dram.ap().rearrange("(a p) e -> p a e", p=128)
            )
            for eb in range(EB):
                ps = ppsum.tile([128, S], F32, name=f"{name}_tps{eb}", tag="pps")
                for sb in range(SB):
                    nc.tensor.transpose(
                        ps[:, sb * 128 : (sb + 1) * 128],
                        xn[:, sb, eb * 128 : (eb + 1) * 128],
                        identity,
                    )
                if eb % 2 == 0:
                    nc.scalar.copy(xT[:, eb, :], ps)
                else:
                    nc.vector.tensor_copy(xT[:, eb, :], ps)
            xT_tiles[name] = xT

        wT_tiles = {}
        for name, dram in (("Wq", Wq), ("Wk", Wk), ("Wv", Wv)):
            wdt = MM_DT if name == "Wv" else BF16
            wT = proT.tile([128, EB, E], wdt, name=f"{name}T")
            wn = pro.tile([128, EB, E], F32, name=f"{name}_nat", tag="wnat")
            nc.sync.dma_start(
                out=wn, in_=dram.ap().rearrange("(a p) e -> p a e", p=128)
            )
            for eb in range(EB):
                ps = ppsum.tile([128, E], F32, name=f"{name}_tps{eb}", tag="pps")
                for rb in range(EB):
                    nc.tensor.transpose(
                        ps[:, rb * 128 : (rb + 1) * 128],
                        wn[:, rb, eb * 128 : (eb + 1) * 128],
                        identity,
                    )
                if eb % 2 == 0:
                    nc.scalar.copy(wT[:, eb, :], ps)
                else:
                    nc.vector.tensor_copy(wT[:, eb, :], ps)
            wT_tiles[name] = wT

        # q^T/k^T projections: [head-pair p] psum[128, S] = sum_eb WxT[:,eb,p-cols].T @ xT[:,eb,:]
        for dst, wname, xname in ((qT_all, "Wq", "query"), (kT_all, "Wk", "key")):
            wT, xT = wT_tiles[wname], xT_tiles[xname]
            for p in range(EB):
                ps = ppsum.tile([128, S], F32, name=f"{wname}p{p}", tag="pps")
                for eb in range(EB):
                    for nh in range(2):
                        nc.tensor.matmul(
                            ps[:, nh * 512 : (nh + 1) * 512],
                            lhsT=wT[:, eb, p * 128 : (p + 1) * 128],
                            rhs=xT[:, eb, nh * 512 : (nh + 1) * 512],
                            start=(eb == 0),
                            stop=(eb == EB - 1),
                        )
                if p % 2 == 0:
                    nc.scalar.copy(dst[:, p, :], ps)
                else:
                    nc.vector.tensor_copy(dst[:, p, :], ps)

        # v natural: [s-block m] psum[128, E] = sum_eb valueT[:,eb,m-cols].T @ WvT[:,eb,:]
        wvT, valT = wT_tiles["Wv"], xT_tiles["value"]
        for m in range(SB):
            ps = ppsum.tile([128, E], F32, name=f"vp{m}", tag="pps")
            for eb in range(EB):
                nc.tensor.matmul(
                    ps,
                    lhsT=valT[:, eb, m * 128 : (m + 1) * 128],
                    rhs=wvT[:, eb, :],
                    start=(eb == 0),
                    stop=(eb == EB - 1),
                )
            if m % 2 == 0:
                nc.scalar.copy(vp[:, m, :], ps)
            else:
                nc.vector.tensor_copy(vp[:, m, :], ps)

    # --- main loop -------------------------------------------------------
    with (
        tc.tile_pool(name="scps", bufs=2, space="PSUM") as scps_pool,
        tc.tile_pool(name="trps", bufs=1, space="PSUM") as trps_pool,
        tc.tile_pool(name="otps", bufs=1, space="PSUM") as otps_pool,
        tc.tile_pool(name="mask", bufs=2) as mask_pool,
        tc.tile_pool(name="cexp", bufs=3) as cexp_pool,
        tc.tile_pool(name="pexp", bufs=3) as pexp_pool,
        tc.tile_pool(name="t2", bufs=2) as t2_pool,
        tc.tile_pool(name="attn", bufs=3) as attn_pool,
        tc.tile_pool(name="attnT", bufs=2) as attnT_pool,
        tc.tile_pool(name="osb", bufs=2) as osb_pool,
        tc.tile_pool(name="small", bufs=24) as small,
    ):
        for h in range(H):
            p, po = h // 2, 64 * (h % 2)
            attnT = attnT_pool.tile([128, SB, S], MM_DT, name=f"attnT{h}", tag="attnT")
            mhalf = {}
            for g2 in range(2):
                mh = mask_pool.tile([128, 4, S], F32, name=f"m{h}_{g2}", tag="mask")
                nc.sync.dma_start(
                    out=mh,
                    in_=mask_ap[h, g2 * 512 : (g2 + 1) * 512, :].rearrange(
                        "(a p) k -> p a k", p=128
                    ),
                )
                mhalf[g2] = mh
            for qb in range(SB):
                qsl = slice(qb * 128, (qb + 1) * 128)
                mt = mhalf[qb // 4][:, qb % 4, :]
                # scores
                sc = scps_pool.tile([128, S], F32, name=f"sc{h}_{qb}", tag="sc")
                for nh in range(2):
                    nc.tensor.matmul(
                        sc[:, nh * 512 : (nh + 1) * 512],
                        lhsT=qT_all[po : po + D, p, qsl],
                        rhs=kT_all[po : po + D, p, nh * 512 : (nh + 1) * 512],
                        start=True,
                        stop=True,
                    )
                # exp + row sums
                cexp = cexp_pool.tile([128, S], F32, name=f"ce{h}_{qb}", tag="cexp")
                sums = small.tile([128, 2], F32, name=f"su{h}_{qb}", tag="sums")
                nc.scalar.activation(cexp, sc, AF.Exp, scale=SCALE, accum_out=sums[:, 0:1])
                pexp = pexp_pool.tile([128, S], F32, name=f"pe{h}_{qb}", tag="pexp")
                nc.scalar.activation(pexp, mt, AF.Exp, accum_out=sums[:, 1:2])
                # per-row gate coefficients
                rec = small.tile([128, 2], F32, name=f"re{h}_{qb}", tag="rec")
                nc.vector.reciprocal(rec, sums)
                arow = small.tile([128, 1], F32, name=f"ar{h}_{qb}", tag="ar")
                nc.vector.tensor_scalar(arow, rec[:, 0:1], gbc[:, H + h : H + h + 1], None, ALU.mult)
                brow = small.tile([128, 1], F32, name=f"br{h}_{qb}", tag="br")
                nc.vector.tensor_scalar(brow, rec[:, 1:2], gbc[:, h : h + 1], None, ALU.mult)
                # attn = cexp*arow + pexp*brow
                t2 = t2_pool.tile([128, S], F32, name=f"t2{h}_{qb}", tag="t2")
                nc.vector.tensor_scalar(t2, pexp, brow, None, ALU.mult)
                attn_sb = attn_pool.tile([128, S], MM_DT, name=f"at{h}_{qb}", tag="attn")
                nc.vector.scalar_tensor_tensor(
                    attn_sb, cexp, arow, t2, ALU.mult, ALU.add
                )
                nc.sync.dma_start(out=attn_out.ap()[h, qsl, :], in_=attn_sb.bitcast(F32))
                # transpose attn -> attn_T
                tps = trps_pool.tile([128, S], MM_DT, name=f"tp{h}_{qb}", tag="tps")
                for kb in range(SB):
                    nc.tensor.transpose(
                        tps[:, kb * 128 : (kb + 1) * 128],
                        attn_sb[:, kb * 128 : (kb + 1) * 128],
                        identity_r,
                    )
                src3 = tps.rearrange("p (k q) -> p k q", k=SB)
                dst3 = attnT[:, :, qsl]
                if qb % 2 == 0:
                    nc.scalar.copy(dst3, src3)
                else:
                    nc.vector.tensor_copy(dst3, src3)
            # out^T = sum over k-blocks of v-block.T @ attn_T-block
            ot = otps_pool.tile([64, S], F32, name=f"ot{h}", tag="ot")
            for kb in range(SB):
                for nh in range(2):
                    nc.tensor.matmul(
                        ot[:, nh * 512 : (nh + 1) * 512],
                        lhsT=vp[:, kb, h * D : (h + 1) * D],
                        rhs=attnT[:, kb, nh * 512 : (nh + 1) * 512],
                        start=(kb == 0),
                        stop=(kb == SB - 1),
                    )
            osb = osb_pool.tile([64, S], F32, name=f"ot_sb{h}", tag="osb")
            nc.scalar.copy(osb, ot)
            nc.sync.dma_start(out=out_t.ap()[h], in_=osb)


def _build():
    nc = bacc.Bacc("TRN2", target_bir_lowering=False, debug=False)
    query = nc.dram_tensor("query", [S, E], F32, kind="ExternalInput")
    key = nc.dram_tensor("key", [S, E], F32, kind="ExternalInput")
    value = nc.dram_tensor("value", [S, E], F32, kind="ExternalInput")
    Wq = nc.dram_tensor("Wq", [E, E], F32, kind="ExternalInput")
    Wk = nc.dram_tensor("Wk", [E, E], F32, kind="ExternalInput")
    Wv = nc.dram_tensor("Wv", [E, E], F32, kind="ExternalInput")
    gating = nc.dram_tensor("gating", [H], F32, kind="ExternalInput")
    mask = nc.dram_tensor("mask", [H, S, S], F32, kind="ExternalInput")
    attn_out = nc.dram_tensor("attn_out", [H, S, S], F32, kind="ExternalOutput")
    out_t = nc.dram_tensor("out_t", [H, D, S], F32, kind="ExternalOutput")

    from contextlib import ExitStack

    with tile.TileContext(nc) as tc, ExitStack() as ctx:
        _emit(
            nc, tc, ctx,
            (query, key, value, Wq, Wk, Wv, gating, mask, attn_out, out_t),
        )
    nc.compile()
    return nc


def kernel(query, key, value, attn_mask, Wq, Wk, Wv, gating_param):
    global LAST_RESULTS
    query = np.ascontiguousarray(np.asarray(query, dtype=np.float32))
    key = np.ascontiguousarray(np.asarray(key, dtype=np.float32))
    value = np.ascontiguousarray(np.asarray(value, dtype=np.float32))
    attn_mask = np.ascontiguousarray(np.asarray(attn_mask, dtype=np.float32))
    Wq = np.ascontiguousarray(np.asarray(Wq, dtype=np.float32))
    Wk = np.ascontiguousarray(np.asarray(Wk, dtype=np.float32))
    Wv = np.ascontiguousarray(np.asarray(Wv, dtype=np.float32))
    gating_param = np.ascontiguousarray(np.asarray(gating_param, dtype=np.float32))
    B = query.shape[0]

    nc = _build()

    mask4 = attn_mask.reshape(B, H, S, S)
    in_maps = []
    for c in range(B):
        in_maps.append(
            {
                "query": query[c],
                "key": key[c],
                "value": value[c],
                "Wq": Wq,
                "Wk": Wk,
                "Wv": Wv,
                "gating": gating_param,
                "mask": mask4[c],
            }
        )

    trace = bool(int(os.environ.get("KERNEL_TRACE", "0")))
    res = run_bass_kernel_spmd(
        nc, in_maps, core_ids=list(range(NCORES)), trace=trace
    )
    LAST_RESULTS = res

    attn = np.stack([res.results[c]["attn_out"] for c in range(B)], axis=0)
    out_t = np.stack([res.results[c]["out_t"] for c in range(B)], axis=0)
    # out[b, s, h*64+d] = out_t[b, h, d, s]
    out = out_t.transpose(0, 3, 1, 2).reshape(B, S, E)
    return out, attn


# revision 28
# speedup vs baseline: 1.0009x; 1.0009x over previous
"""Trainium2 Bass kernel for gated relative-position attention.

Problem (hardcoded shapes): B=8, S=1024, E=512, H=8 heads, D=64.
    q = query @ Wq.T ; k = key @ Wk.T ; v = value @ Wv.T      (per-head split)
    content = softmax(q k^T / sqrt(E)) ; pos = softmax(mask)
    attn = (1-sigmoid(g)) * content + sigmoid(g) * pos ; attn /= attn.sum(-1)
    out  = attn @ v
Returns (out [B,S,E], attn [B,H,S,S]).

Sharding: data parallel over batch — core c computes batch element c
(all 8 heads). No collectives; host scatters inputs / gathers outputs.

Per-core pipeline (all fp32):
  prologue: PE-transpose query/key/value/W to get X^T layouts, project
            q^T,k^T (head-dim on partitions) and v (natural), sigmoid gates.
  per (head, q-block of 128 rows):
    PE:  scores[128,1024] = q^T.T @ k^T           (K=D=64)
    ACT: cexp = Exp(scale*scores)  with accum_out -> row sums (free)
    ACT: pexp = Exp(mask tile)     with accum_out -> row sums
    DVE: a_row=(1-g)/csum, b_row=g/psum  (per-partition scalars)
    DVE: attn = (cexp*a_row) + (pexp*b_row)   [tensor_scalar + scalar_tensor_tensor]
    PE:  transpose attn 128x128 blocks -> attn_T ; ACT/DVE copy PSUM->SBUF
  per head: PE: out^T[64,1024] = sum_k v_k^T-block matmuls over attn_T
  (renormalization by attn.sum(-1) is skipped: both softmaxes sum to 1,
   so the sum is 1 +- ~1e-6 and the division is a no-op at fp32 scale)

Host gathers attn directly and transposes out^T -> out (numpy).
"""

import os

os.environ.setdefault("MYCRO_LOCAL_CACHE", "1")

import numpy as np

import concourse.bass as bass
import concourse.mybir as mybir
import concourse.tile as tile
from concourse import bacc
from concourse.bass_utils import run_bass_kernel_spmd
from concourse.masks import make_identity

S = 1024
E = 512
H = 8
D = 64
NCORES = 8
SCALE = float(E) ** -0.5
F32 = mybir.dt.float32
F32R = mybir.dt.float32r
BF16 = mybir.dt.bfloat16
USE_F32R = os.environ.get("KERNEL_F32R", "1") == "1"
MM_DT = F32R if USE_F32R else F32
AF = mybir.ActivationFunctionType
ALU = mybir.AluOpType

SB = S // 128  # 8 s-blocks
EB = E // 128  # 4 e-blocks

LAST_RESULTS = None  # BassKernelResults of the most recent run (for test.py)


def _emit(nc, tc, ctx, tensors):
    query, key, value, Wq, Wk, Wv, gating, mask, attn_out, out_t = tensors

    pers = ctx.enter_context(tc.tile_pool(name="pers", bufs=1))

    # --- constants -------------------------------------------------------
    identity = pers.tile([128, 128], F32)
    make_identity(nc, identity)
    identity_r = pers.tile([128, 128], MM_DT)
    nc.vector.tensor_copy(identity_r, identity)
    ones1 = pers.tile([1, 128], F32)
    nc.vector.memset(ones1, 1.0)

    # --- gates: sigmoid via exp + reciprocal (stays in exp table set) ----
    # broadcast gating to all 128 partitions via 0-stride DMA, then compute
    # sigmoid replicated per partition (no PSUM / matmul involved).
    g_in = pers.tile([128, H], F32)
    nc.sync.dma_start(out=g_in, in_=gating.ap().partition_broadcast(128))
    g_eneg = pers.tile([128, H], F32)
    nc.scalar.activation(g_eneg, g_in, AF.Exp, scale=-1.0)
    g_den = pers.tile([128, H], F32)
    nc.vector.tensor_scalar(g_den, g_eneg, 1.0, None, ALU.add)
    gbc = pers.tile([128, 3 * H], F32)  # cols 0..7 = g, 8..15 = 1-g, 16..23 = g/(1-g)
    nc.vector.reciprocal(gbc[:, 0:H], g_den)
    nc.vector.tensor_scalar(gbc[:, H : 2 * H], gbc[:, 0:H], -1.0, 1.0, ALU.mult, ALU.add)
    grat = pers.tile([128, H], F32)
    nc.vector.reciprocal(grat, gbc[:, H : 2 * H])
    nc.vector.tensor_tensor(gbc[:, 2 * H : 3 * H], gbc[:, 0:H], grat, ALU.mult)

    # --- prologue: transposes + projections ------------------------------
    qT_all = pers.tile([128, EB, S], BF16)  # (q W^T)^T: head-dim on partitions
    kT_all = pers.tile([128, EB, S], BF16)
    vp = pers.tile([128, SB, E], MM_DT)      # v natural: [s-block partitions, h*64+d]


    mask_ap = mask.ap()

    with (
        tc.tile_pool(name="pro", bufs=2) as pro,
        tc.tile_pool(name="proT", bufs=1) as proT,
        tc.tile_pool(name="propsum", bufs=2, space="PSUM") as ppsum,
    ):
        # sacrificial transpose: absorbs the gpsimd(identity) wait on PE so
        # later PE instructions don't each need a Pool semaphore wait.
        warm = ppsum.tile([128, S], F32, name="warm", tag="pps")
        nc.tensor.transpose(warm[:, 0:128], identity, identity)

        xT_tiles = {}
        for name, dram in (("query", query), ("key", key), ("value", value)):
            xdt = MM_DT if name == "value" else BF16
            xT = proT.tile([128, EB, S], xdt, name=f"{name}T")
            # one DMA: [s, e] -> [s%128, s//128, e]
            xn = pro.tile([128, SB, E], F32, name=f"{name}_nat", tag="xnat")
            nc.sync.dma_start(
                out=xn, in_=dram.ap().rearrange("(a p) e -> p a e", p=128)
            )
            for eb in range(EB):
                ps = ppsum.tile([128, S], F32, name=f"{name}_tps{eb}", tag="pps")
                for sb in range(SB):
                    nc.tensor.transpose(
                        ps[:, sb * 128 : (sb + 1) * 128],
                        xn[:, sb, eb * 128 : (eb + 1) * 128],
                        identity,
                    )
                if eb % 2 == 0:
                    nc.scalar.copy(xT[:, eb, :], ps)
                else:
                    nc.vector.tensor_copy(xT[:, eb, :], ps)
            xT_tiles[name] = xT

        wT_tiles = {}
        for name, dram in (("Wq", Wq), ("Wk", Wk), ("Wv", Wv)):
            wdt = MM_DT if name == "Wv" else BF16
            wT = proT.tile([128, EB, E], wdt, name=f"{name}T")
            wn = pro.tile([128, EB, E], F32, name=f"{name}_nat", tag="wnat")
            nc.sync.dma_start(
                out=wn, in_=dram.ap().rearrange("(a p) e -> p a e", p=128)
            )
            for eb in range(EB):
                ps = ppsum.tile([128, E], F32, name=f"{name}_tps{eb}", tag="pps")
                for rb in range(EB):
                    nc.tensor.transpose(
                        ps[:, rb * 128 : (rb + 1) * 128],
                        wn[:, rb, eb * 128 : (eb + 1) * 128],
                        identity,
                    )
                if eb % 2 == 0:
                    nc.scalar.copy(wT[:, eb, :], ps)
                else:
                    nc.vector.tensor_copy(wT[:, eb, :], ps)
            wT_tiles[name] = wT

        # q^T/k^T projections: [head-pair p] psum[128, S] = sum_eb WxT[:,eb,p-cols].T @ xT[:,eb,:]
        for dst, wname, xname in ((qT_all, "Wq", "query"), (kT_all, "Wk", "key")):
            wT, xT = wT_tiles[wname], xT_tiles[xname]
            for p in range(EB):
                ps = ppsum.tile([128, S], F32, name=f"{wname}p{p}", tag="pps")
                for eb in range(EB):
                    for nh in range(2):
                        nc.tensor.matmul(
                            ps[:, nh * 512 : (nh + 1) * 512],
                            lhsT=wT[:, eb, p * 128 : (p + 1) * 128],
                            rhs=xT[:, eb, nh * 512 : (nh + 1) * 512],
                            start=(eb == 0),
                            stop=(eb == EB - 1),
                        )
                if p % 2 == 0:
                    nc.scalar.copy(dst[:, p, :], ps)
                else:
                    nc.vector.tensor_copy(dst[:, p, :], ps)

        # v natural: [s-block m] psum[128, E] = sum_eb valueT[:,eb,m-cols].T @ WvT[:,eb,:]
        wvT, valT = wT_tiles["Wv"], xT_tiles["value"]
        for m in range(SB):
            ps = ppsum.tile([128, E], F32, name=f"vp{m}", tag="pps")
            for eb in range(EB):
                nc.tensor.matmul(
                    ps,
                    lhsT=valT[:, eb, m * 128 : (m + 1) * 128],
                    rhs=wvT[:, eb, :],
                    start=(eb == 0),
                    stop=(eb == EB - 1),
                )
            if m % 2 == 0:
                nc.scalar.copy(vp[:, m, :], ps)
            else:
                nc.vector.tensor_copy(vp[:, m, :], ps)

    # --- main loop -------------------------------------------------------
    with (
        tc.tile_pool(name="scps", bufs=2, space="PSUM") as scps_pool,
        tc.tile_pool(name="trps", bufs=1, space="PSUM") as trps_pool,
        tc.tile_pool(name="otps", bufs=1, space="PSUM") as otps_pool,
        tc.tile_pool(name="mask", bufs=3) as mask_pool,
        tc.tile_pool(name="cexp", bufs=4) as cexp_pool,
        tc.tile_pool(name="pexp", bufs=3) as pexp_pool,
        tc.tile_pool(name="t2", bufs=2) as t2_pool,
        tc.tile_pool(name="attn", bufs=3) as attn_pool,
        tc.tile_pool(name="attnT", bufs=2) as attnT_pool,
        tc.tile_pool(name="osb", bufs=2) as osb_pool,
        tc.tile_pool(name="small", bufs=24) as small,
    ):
        for h in range(H):
            p, po = h // 2, 64 * (h % 2)
            attnT = attnT_pool.tile([128, SB, S], MM_DT, name=f"attnT{h}", tag="attnT")
            mhalf = {}
            for g2 in range(2):
                mh = mask_pool.tile([128, 4, S], F32, name=f"m{h}_{g2}", tag="mask")
                nc.sync.dma_start(
                    out=mh,
                    in_=mask_ap[h, g2 * 512 : (g2 + 1) * 512, :].rearrange(
                        "(a p) k -> p a k", p=128
                    ),
                )
                mhalf[g2] = mh
            for qb in range(SB):
                qsl = slice(qb * 128, (qb + 1) * 128)
                mt = mhalf[qb // 4][:, qb % 4, :]
                # scores
                sc = scps_pool.tile([128, S], F32, name=f"sc{h}_{qb}", tag="sc")
                for nh in range(2):
                    nc.tensor.matmul(
                        sc[:, nh * 512 : (nh + 1) * 512],
                        lhsT=qT_all[po : po + D, p, qsl],
                        rhs=kT_all[po : po + D, p, nh * 512 : (nh + 1) * 512],
                        start=True,
                        stop=True,
                    )
                # exp + row sums
                cexp = cexp_pool.tile([128, S], F32, name=f"ce{h}_{qb}", tag="cexp")
                sums = small.tile([128, 2], F32, name=f"su{h}_{qb}", tag="sums")
                nc.scalar.activation(cexp, sc, AF.Exp, scale=SCALE, accum_out=sums[:, 0:1])
                pexp = pexp_pool.tile([128, S], F32, name=f"pe{h}_{qb}", tag="pexp")
                nc.scalar.activation(pexp, mt, AF.Exp, accum_out=sums[:, 1:2])
                # per-row gate coefficients
                rec = small.tile([128, 2], F32, name=f"re{h}_{qb}", tag="rec")
                nc.vector.reciprocal(rec, sums)
                arow = small.tile([128, 1], F32, name=f"ar{h}_{qb}", tag="ar")
                nc.vector.tensor_scalar(arow, rec[:, 0:1], gbc[:, H + h : H + h + 1], None, ALU.mult)
                brow = small.tile([128, 1], F32, name=f"br{h}_{qb}", tag="br")
                nc.vector.tensor_scalar(brow, rec[:, 1:2], gbc[:, h : h + 1], None, ALU.mult)
                # attn = cexp*arow + pexp*brow
                t2 = t2_pool.tile([128, S], F32, name=f"t2{h}_{qb}", tag="t2")
                nc.vector.tensor_scalar(t2, pexp, brow, None, ALU.mult)
                attn_sb = attn_pool.tile([128, S], MM_DT, name=f"at{h}_{qb}", tag="attn")
                nc.vector.scalar_tensor_tensor(
                    attn_sb, cexp, arow, t2, ALU.mult, ALU.add
                )
                nc.sync.dma_start(out=attn_out.ap()[h, qsl, :], in_=attn_sb.bitcast(F32))
                # transpose attn -> attn_T
                tps = trps_pool.tile([128, S], MM_DT, name=f"tp{h}_{qb}", tag="tps")
                for kb in range(SB):
                    nc.tensor.transpose(
                        tps[:, kb * 128 : (kb + 1) * 128],
                        attn_sb[:, kb * 128 : (kb + 1) * 128],
                        identity_r,
                    )
                src3 = tps.rearrange("p (k q) -> p k q", k=SB)
                dst3 = attnT[:, :, qsl]
                if qb % 2 == 0:
                    nc.scalar.copy(dst3, src3)
                else:
                    nc.vector.tensor_copy(dst3, src3)
            # out^T = sum over k-blocks of v-block.T @ attn_T-block
            ot = otps_pool.tile([64, S], F32, name=f"ot{h}", tag="ot")
            for kb in range(SB):
                for nh in range(2):
                    nc.tensor.matmul(
                        ot[:, nh * 512 : (nh + 1) * 512],
                        lhsT=vp[:, kb, h * D : (h + 1) * D],
                        rhs=attnT[:, kb, nh * 512 : (nh + 1) * 512],
                        start=(kb == 0),
                        stop=(kb == SB - 1),
                    )
            osb = osb_pool.tile([64, S], F32, name=f"ot_sb{h}", tag="osb")
            nc.scalar.copy(osb, ot)
            nc.sync.dma_start(out=out_t.ap()[h], in_=osb)


def _build():
    nc = bacc.Bacc("TRN2", target_bir_lowering=False, debug=False)
    query = nc.dram_tensor("query", [S, E], F32, kind="ExternalInput")
    key = nc.dram_tensor("key", [S, E], F32, kind="ExternalInput")
    value = nc.dram_tensor("value", [S, E], F32, kind="ExternalInput")
    Wq = nc.dram_tensor("Wq", [E, E], F32, kind="ExternalInput")
    Wk = nc.dram_tensor("Wk", [E, E], F32, kind="ExternalInput")
    Wv = nc.dram_tensor("Wv", [E, E], F32, kind="ExternalInput")
    gating = nc.dram_tensor("gating", [H], F32, kind="ExternalInput")
    mask = nc.dram_tensor("mask", [H, S, S], F32, kind="ExternalInput")
    attn_out = nc.dram_tensor("attn_out", [H, S, S], F32, kind="ExternalOutput")
    out_t = nc.dram_tensor("out_t", [H, D, S], F32, kind="ExternalOutput")

    from contextlib import ExitStack

    with tile.TileContext(nc) as tc, ExitStack() as ctx:
        _emit(
            nc, tc, ctx,
            (query, key, value, Wq, Wk, Wv, gating, mask, attn_out, out_t),
        )
    nc.compile()
    return nc


def kernel(query, key, value, attn_mask, Wq, Wk, Wv, gating_param):
    global LAST_RESULTS
    query = np.ascontiguousarray(np.asarray(query, dtype=np.float32))
    key = np.ascontiguousarray(np.asarray(key, dtype=np.float32))
    value = np.ascontiguousarray(np.asarray(value, dtype=np.float32))
    attn_mask = np.ascontiguousarray(np.asarray(attn_mask, dtype=np.float32))
    Wq = np.ascontiguousarray(np.asarray(Wq, dtype=np.float32))
    Wk = np.ascontiguousarray(np.asarray(Wk, dtype=np.float32))
    Wv = np.ascontiguousarray(np.asarray(Wv, dtype=np.float32))
    gating_param = np.ascontiguousarray(np.asarray(gating_param, dtype=np.float32))
    B = query.shape[0]

    nc = _build()

    mask4 = attn_mask.reshape(B, H, S, S)
    in_maps = []
    for c in range(B):
        in_maps.append(
            {
                "query": query[c],
                "key": key[c],
                "value": value[c],
                "Wq": Wq,
                "Wk": Wk,
                "Wv": Wv,
                "gating": gating_param,
                "mask": mask4[c],
            }
        )

    trace = bool(int(os.environ.get("KERNEL_TRACE", "0")))
    res = run_bass_kernel_spmd(
        nc, in_maps, core_ids=list(range(NCORES)), trace=trace
    )
    LAST_RESULTS = res

    attn = np.stack([res.results[c]["attn_out"] for c in range(B)], axis=0)
    out_t = np.stack([res.results[c]["out_t"] for c in range(B)], axis=0)
    # out[b, s, h*64+d] = out_t[b, h, d, s]
    out = out_t.transpose(0, 3, 1, 2).reshape(B, S, E)
    return out, attn


# revision 33
# speedup vs baseline: 1.0156x; 1.0146x over previous
"""Trainium2 Bass kernel for gated relative-position attention.

Problem (hardcoded shapes): B=8, S=1024, E=512, H=8 heads, D=64.
    q = query @ Wq.T ; k = key @ Wk.T ; v = value @ Wv.T      (per-head split)
    content = softmax(q k^T / sqrt(E)) ; pos = softmax(mask)
    attn = (1-sigmoid(g)) * content + sigmoid(g) * pos ; attn /= attn.sum(-1)
    out  = attn @ v
Returns (out [B,S,E], attn [B,H,S,S]).

Sharding: data parallel over batch — core c computes batch element c
(all 8 heads). No collectives; host scatters inputs / gathers outputs.

Per-core pipeline (all fp32):
  prologue: PE-transpose query/key/value/W to get X^T layouts, project
            q^T,k^T (head-dim on partitions) and v (natural), sigmoid gates.
  per (head, q-block of 128 rows):
    PE:  scores[128,1024] = q^T.T @ k^T           (K=D=64)
    ACT: cexp = Exp(scale*scores)  with accum_out -> row sums (free)
    ACT: pexp = Exp(mask tile)     with accum_out -> row sums
    DVE: a_row=(1-g)/csum, b_row=g/psum  (per-partition scalars)
    DVE: attn = (cexp*a_row) + (pexp*b_row)   [tensor_scalar + scalar_tensor_tensor]
    PE:  transpose attn 128x128 blocks -> attn_T ; ACT/DVE copy PSUM->SBUF
  per head: PE: out^T[64,1024] = sum_k v_k^T-block matmuls over attn_T
  (renormalization by attn.sum(-1) is skipped: both softmaxes sum to 1,
   so the sum is 1 +- ~1e-6 and the division is a no-op at fp32 scale)

Host gathers attn directly and transposes out^T -> out (numpy).
"""

import os

os.environ.setdefault("MYCRO_LOCAL_CACHE", "1")

import numpy as np

import concourse.bass as bass
import concourse.mybir as mybir
import concourse.tile as tile
from concourse import bacc
from concourse.bass_utils import run_bass_kernel_spmd
from concourse.masks import make_identity

S = 1024
E = 512
H = 8
D = 64
NCORES = 8
SCALE = float(E) ** -0.5
F32 = mybir.dt.float32
F32R = mybir.dt.float32r
BF16 = mybir.dt.bfloat16
USE_F32R = os.environ.get("KERNEL_F32R", "1") == "1"
MM_DT = F32R if USE_F32R else F32
AF = mybir.ActivationFunctionType
ALU = mybir.AluOpType

SB = S // 128  # 8 s-blocks
EB = E // 128  # 4 e-blocks

LAST_RESULTS = None  # BassKernelResults of the most recent run (for test.py)


def _emit(nc, tc, ctx, tensors):
    query, key, value, Wq, Wk, Wv, gating, mask, attn_out, out_t = tensors

    pers = ctx.enter_context(tc.tile_pool(name="pers", bufs=1))

    # --- constants -------------------------------------------------------
    identity = pers.tile([128, 128], F32)
    make_identity(nc, identity)
    identity_r = pers.tile([128, 128], MM_DT)
    nc.vector.tensor_copy(identity_r, identity)
    ones1 = pers.tile([1, 128], F32)
    nc.vector.memset(ones1, 1.0)

    # --- gates: sigmoid via exp + reciprocal (stays in exp table set) ----
    # broadcast gating to all 128 partitions via 0-stride DMA, then compute
    # sigmoid replicated per partition (no PSUM / matmul involved).
    g_in = pers.tile([128, H], F32)
    nc.sync.dma_start(out=g_in, in_=gating.ap().partition_broadcast(128))
    g_eneg = pers.tile([128, H], F32)
    nc.scalar.activation(g_eneg, g_in, AF.Exp, scale=-1.0)
    g_den = pers.tile([128, H], F32)
    nc.vector.tensor_scalar(g_den, g_eneg, 1.0, None, ALU.add)
    gbc = pers.tile([128, 3 * H], F32)  # cols 0..7 = g, 8..15 = 1-g, 16..23 = g/(1-g)
    nc.vector.reciprocal(gbc[:, 0:H], g_den)
    nc.vector.tensor_scalar(gbc[:, H : 2 * H], gbc[:, 0:H], -1.0, 1.0, ALU.mult, ALU.add)
    grat = pers.tile([128, H], F32)
    nc.vector.reciprocal(grat, gbc[:, H : 2 * H])
    nc.vector.tensor_tensor(gbc[:, 2 * H : 3 * H], gbc[:, 0:H], grat, ALU.mult)

    # --- prologue: transposes + projections ------------------------------
    qT_all = pers.tile([128, EB, S], BF16)  # (q W^T)^T: head-dim on partitions
    kT_all = pers.tile([128, EB, S], BF16)
    vp = pers.tile([128, SB, E], MM_DT)      # v natural: [s-block partitions, h*64+d]


    mask_ap = mask.ap()

    with (
        tc.tile_pool(name="pro", bufs=2) as pro,
        tc.tile_pool(name="proT", bufs=1) as proT,
        tc.tile_pool(name="propsum", bufs=2, space="PSUM") as ppsum,
    ):
        # sacrificial transpose: absorbs the gpsimd(identity) wait on PE so
        # later PE instructions don't each need a Pool semaphore wait.
        warm = ppsum.tile([128, S], F32, name="warm", tag="pps")
        nc.tensor.transpose(warm[:, 0:128], identity, identity)

        xT_tiles = {}
        for name, dram in (("query", query), ("key", key), ("value", value)):
            xdt = MM_DT if name == "value" else BF16
            xT = proT.tile([128, EB, S], xdt, name=f"{name}T")
            # one DMA: [s, e] -> [s%128, s//128, e]
            xn = pro.tile([128, SB, E], F32, name=f"{name}_nat", tag="xnat")
            nc.sync.dma_start(
                out=xn, in_=dram.ap().rearrange("(a p) e -> p a e", p=128)
            )
            for eb in range(EB):
                ps = ppsum.tile([128, S], F32, name=f"{name}_tps{eb}", tag="pps")
                for sb in range(SB):
                    nc.tensor.transpose(
                        ps[:, sb * 128 : (sb + 1) * 128],
                        xn[:, sb, eb * 128 : (eb + 1) * 128],
                        identity,
                    )
                if eb % 2 == 0:
                    nc.scalar.copy(xT[:, eb, :], ps)
                else:
                    nc.vector.tensor_copy(xT[:, eb, :], ps)
            xT_tiles[name] = xT

        wT_tiles = {}
        for name, dram in (("Wq", Wq), ("Wk", Wk), ("Wv", Wv)):
            wdt = MM_DT if name == "Wv" else BF16
            wT = proT.tile([128, EB, E], wdt, name=f"{name}T")
            wn = pro.tile([128, EB, E], F32, name=f"{name}_nat", tag="wnat")
            nc.sync.dma_start(
                out=wn, in_=dram.ap().rearrange("(a p) e -> p a e", p=128)
            )
            for eb in range(EB):
                ps = ppsum.tile([128, E], F32, name=f"{name}_tps{eb}", tag="pps")
                for rb in range(EB):
                    nc.tensor.transpose(
                        ps[:, rb * 128 : (rb + 1) * 128],
                        wn[:, rb, eb * 128 : (eb + 1) * 128],
                        identity,
                    )
                if eb % 2 == 0:
                    nc.scalar.copy(wT[:, eb, :], ps)
                else:
                    nc.vector.tensor_copy(wT[:, eb, :], ps)
            wT_tiles[name] = wT

        # q^T/k^T projections: [head-pair p] psum[128, S] = sum_eb WxT[:,eb,p-cols].T @ xT[:,eb,:]
        for dst, wname, xname in ((qT_all, "Wq", "query"), (kT_all, "Wk", "key")):
            wT, xT = wT_tiles[wname], xT_tiles[xname]
            for p in range(EB):
                ps = ppsum.tile([128, S], F32, name=f"{wname}p{p}", tag="pps")
                for eb in range(EB):
                    for nh in range(2):
                        nc.tensor.matmul(
                            ps[:, nh * 512 : (nh + 1) * 512],
                            lhsT=wT[:, eb, p * 128 : (p + 1) * 128],
                            rhs=xT[:, eb, nh * 512 : (nh + 1) * 512],
                            start=(eb == 0),
                            stop=(eb == EB - 1),
                        )
                if p % 2 == 0:
                    nc.scalar.copy(dst[:, p, :], ps)
                else:
                    nc.vector.tensor_copy(dst[:, p, :], ps)

        # v natural: [s-block m] psum[128, E] = sum_eb valueT[:,eb,m-cols].T @ WvT[:,eb,:]
        wvT, valT = wT_tiles["Wv"], xT_tiles["value"]
        for m in range(SB):
            ps = ppsum.tile([128, E], F32, name=f"vp{m}", tag="pps")
            for eb in range(EB):
                nc.tensor.matmul(
                    ps,
                    lhsT=valT[:, eb, m * 128 : (m + 1) * 128],
                    rhs=wvT[:, eb, :],
                    start=(eb == 0),
                    stop=(eb == EB - 1),
                )
            if m % 2 == 0:
                nc.scalar.copy(vp[:, m, :], ps)
            else:
                nc.vector.tensor_copy(vp[:, m, :], ps)

    # --- main loop -------------------------------------------------------
    with (
        tc.tile_pool(name="scps", bufs=2, space="PSUM") as scps_pool,
        tc.tile_pool(name="trps", bufs=1, space="PSUM") as trps_pool,
        tc.tile_pool(name="otps", bufs=1, space="PSUM") as otps_pool,
        tc.tile_pool(name="mask", bufs=3) as mask_pool,
        tc.tile_pool(name="cexp", bufs=4) as cexp_pool,
        tc.tile_pool(name="pexp", bufs=3) as pexp_pool,
        tc.tile_pool(name="t2", bufs=2) as t2_pool,
        tc.tile_pool(name="attn", bufs=3) as attn_pool,
        tc.tile_pool(name="attnT", bufs=2) as attnT_pool,
        tc.tile_pool(name="osb", bufs=2) as osb_pool,
        tc.tile_pool(name="small", bufs=24) as small,
    ):
        for h in range(H):
            p, po = h // 2, 64 * (h % 2)
            attnT = attnT_pool.tile([128, SB, S], MM_DT, name=f"attnT{h}", tag="attnT")
            mhalf = {}
            for g2 in range(2):
                mh = mask_pool.tile([128, 4, S], F32, name=f"m{h}_{g2}", tag="mask")
                nc.sync.dma_start(
                    out=mh,
                    in_=mask_ap[h, g2 * 512 : (g2 + 1) * 512, :].rearrange(
                        "(a p) k -> p a k", p=128
                    ),
                )
                mhalf[g2] = mh
            for qb in range(SB):
                qsl = slice(qb * 128, (qb + 1) * 128)
                mt = mhalf[qb // 4][:, qb % 4, :]
                # scores
                sc = scps_pool.tile([128, S], F32, name=f"sc{h}_{qb}", tag="sc")
                for nh in range(2):
                    nc.tensor.matmul(
                        sc[:, nh * 512 : (nh + 1) * 512],
                        lhsT=qT_all[po : po + D, p, qsl],
                        rhs=kT_all[po : po + D, p, nh * 512 : (nh + 1) * 512],
                        start=True,
                        stop=True,
                    )
                # exp + row sums
                cexp = cexp_pool.tile([128, S], F32, name=f"ce{h}_{qb}", tag="cexp")
                sums = small.tile([128, 2], F32, name=f"su{h}_{qb}", tag="sums")
                nc.scalar.activation(cexp, sc, AF.Exp, scale=SCALE, accum_out=sums[:, 0:1])
                pexp = pexp_pool.tile([128, S], F32, name=f"pe{h}_{qb}", tag="pexp")
                nc.scalar.activation(pexp, mt, AF.Exp, accum_out=sums[:, 1:2])
                # per-row gate coefficients
                rec = small.tile([128, 2], F32, name=f"re{h}_{qb}", tag="rec")
                nc.vector.reciprocal(rec, sums)
                arow = small.tile([128, 1], F32, name=f"ar{h}_{qb}", tag="ar")
                nc.vector.tensor_scalar(arow, rec[:, 0:1], gbc[:, H + h : H + h + 1], None, ALU.mult)
                brow = small.tile([128, 1], F32, name=f"br{h}_{qb}", tag="br")
                nc.vector.tensor_scalar(brow, rec[:, 1:2], gbc[:, h : h + 1], None, ALU.mult)
                # attn = cexp*arow + pexp*brow
                t2 = t2_pool.tile([128, S], F32, name=f"t2{h}_{qb}", tag="t2")
                nc.vector.tensor_scalar(t2, pexp, brow, None, ALU.mult)
                attn_sb = attn_pool.tile([128, S], MM_DT, name=f"at{h}_{qb}", tag="attn")
                nc.vector.scalar_tensor_tensor(
                    attn_sb, cexp, arow, t2, ALU.mult, ALU.add
                )
                nc.sync.dma_start(out=attn_out.ap()[h, qsl, :], in_=attn_sb.bitcast(F32))
                # transpose attn -> attn_T
                tps = trps_pool.tile([128, S], MM_DT, name=f"tp{h}_{qb}", tag="tps")
                for kb in range(SB):
                    nc.tensor.transpose(
                        tps[:, kb * 128 : (kb + 1) * 128],
                        attn_sb[:, kb * 128 : (kb + 1) * 128],
                        identity_r,
                    )
                src3 = tps.rearrange("p (k q) -> p k q", k=SB)
                dst3 = attnT[:, :, qsl]
                if qb % 2 == 0:
                    nc.scalar.copy(dst3, src3)
                else:
                    nc.vector.tensor_copy(dst3, src3)
            # out^T = sum over k-blocks of v-block.T @ attn_T-block
            ot = otps_pool.tile([64, S], F32, name=f"ot{h}", tag="ot")
            for kb in range(SB):
                for nh in range(2):
                    nc.tensor.matmul(
                        ot[:, nh * 512 : (nh + 1) * 512],
                        lhsT=vp[:, kb, h * D : (h + 1) * D],
                        rhs=attnT[:, kb, nh * 512 : (nh + 1) * 512],
                        start=(kb == 0),
                        stop=(kb == SB - 1),
                    )
            osb = osb_pool.tile([64, S], F32, name=f"ot_sb{h}", tag="osb")
            nc.scalar.copy(osb, ot)
            nc.sync.dma_start(out=out_t.ap()[h], in_=osb)


def _build():
    nc = bacc.Bacc("TRN2", target_bir_lowering=False, debug=False)
    query = nc.dram_tensor("query", [S, E], F32, kind="ExternalInput")
    key = nc.dram_tensor("key", [S, E], F32, kind="ExternalInput")
    value = nc.dram_tensor("value", [S, E], F32, kind="ExternalInput")
    Wq = nc.dram_tensor("Wq", [E, E], F32, kind="ExternalInput")
    Wk = nc.dram_tensor("Wk", [E, E], F32, kind="ExternalInput")
    Wv = nc.dram_tensor("Wv", [E, E], F32, kind="ExternalInput")
    gating = nc.dram_tensor("gating", [H], F32, kind="ExternalInput")
    mask = nc.dram_tensor("mask", [H, S, S], F32, kind="ExternalInput")
    attn_out = nc.dram_tensor("attn_out", [H, S, S], F32, kind="ExternalOutput")
    out_t = nc.dram_tensor("out_t", [H, D, S], F32, kind="ExternalOutput")

    from contextlib import ExitStack

    with tile.TileContext(nc) as tc, ExitStack() as ctx:
        _emit(
            nc, tc, ctx,
            (query, key, value, Wq, Wk, Wv, gating, mask, attn_out, out_t),
        )
    nc.compile()
    return nc


def kernel(query, key, value, attn_mask, Wq, Wk, Wv, gating_param):
    global LAST_RESULTS
    query = np.ascontiguousarray(np.asarray(query, dtype=np.float32))
    key = np.ascontiguousarray(np.asarray(key, dtype=np.float32))
    value = np.ascontiguousarray(np.asarray(value, dtype=np.float32))
    attn_mask = np.ascontiguousarray(np.asarray(attn_mask, dtype=np.float32))
    Wq = np.ascontiguousarray(np.asarray(Wq, dtype=np.float32))
    Wk = np.ascontiguousarray(np.asarray(Wk, dtype=np.float32))
    Wv = np.ascontiguousarray(np.asarray(Wv, dtype=np.float32))
    gating_param = np.ascontiguousarray(np.asarray(gating_param, dtype=np.float32))
    B = query.shape[0]

    nc = _build()

    mask4 = attn_mask.reshape(B, H, S, S)
    in_maps = []
    for c in range(B):
        in_maps.append(
            {
                "query": query[c],
                "key": key[c],
                "value": value[c],
                "Wq": Wq,
                "Wk": Wk,
                "Wv": Wv,
                "gating": gating_param,
                "mask": mask4[c],
            }
        )

    trace = bool(int(os.environ.get("KERNEL_TRACE", "0")))
    res = run_bass_kernel_spmd(
        nc, in_maps, core_ids=list(range(NCORES)), trace=trace
    )
    LAST_RESULTS = res

    attn = np.stack([res.results[c]["attn_out"] for c in range(B)], axis=0)
    out_t = np.stack([res.results[c]["out_t"] for c in range(B)], axis=0)
    # out[b, s, h*64+d] = out_t[b, h, d, s]
    out = out_t.transpose(0, 3, 1, 2).reshape(B, S, E)
    return out, attn


# revision 35
# speedup vs baseline: 1.0260x; 1.0103x over previous
"""Trainium2 Bass kernel for gated relative-position attention.

Problem (hardcoded shapes): B=8, S=1024, E=512, H=8 heads, D=64.
    q = query @ Wq.T ; k = key @ Wk.T ; v = value @ Wv.T      (per-head split)
    content = softmax(q k^T / sqrt(E)) ; pos = softmax(mask)
    attn = (1-sigmoid(g)) * content + sigmoid(g) * pos ; attn /= attn.sum(-1)
    out  = attn @ v
Returns (out [B,S,E], attn [B,H,S,S]).

Sharding: data parallel over batch — core c computes batch element c
(all 8 heads). No collectives; host scatters inputs / gathers outputs.

Per-core pipeline (fp32 activations; q/k path bf16, v and the attention
matrix float32r for full-rate PE matmuls — max rel err ~3e-4):
  prologue: PE-transpose query/key/value/W to get X^T layouts, project
            q^T,k^T (head-dim on partitions) and v (natural), sigmoid gates.
  per (head, q-block of 128 rows):
    PE:  scores[128,1024] = q^T.T @ k^T           (K=D=64, bf16)
    ACT: cexp = Exp(scale*scores)  with accum_out -> row sums (free)
    ACT: pexp = Exp(mask tile)     with accum_out -> row sums
    DVE: a_row=(1-g)/csum, b_row=g/psum  (per-partition scalars)
    DVE: attn = (cexp*a_row) + (pexp*b_row)   [tensor_scalar + scalar_tensor_tensor]
    PE:  transpose attn 128x128 blocks -> attn_T ; ACT/DVE copy PSUM->SBUF
  per head: PE: out^T[64,1024] = sum_k v_k^T-block matmuls over attn_T
  (renormalization by attn.sum(-1) is skipped: both softmaxes sum to 1,
   so the sum is 1 +- ~1e-6 and the division is a no-op at fp32 scale)

Host gathers attn directly and transposes out^T -> out (numpy).
"""

import os

os.environ.setdefault("MYCRO_LOCAL_CACHE", "1")

import numpy as np

import concourse.bass as bass
import concourse.mybir as mybir
import concourse.tile as tile
from concourse import bacc
from concourse.bass_utils import run_bass_kernel_spmd
from concourse.masks import make_identity

S = 1024
E = 512
H = 8
D = 64
NCORES = 8
SCALE = float(E) ** -0.5
F32 = mybir.dt.float32
F32R = mybir.dt.float32r
BF16 = mybir.dt.bfloat16
USE_F32R = os.environ.get("KERNEL_F32R", "1") == "1"
MM_DT = F32R if USE_F32R else F32
AF = mybir.ActivationFunctionType
ALU = mybir.AluOpType

SB = S // 128  # 8 s-blocks
EB = E // 128  # 4 e-blocks

LAST_RESULTS = None  # BassKernelResults of the most recent run (for test.py)


def _emit(nc, tc, ctx, tensors):
    query, key, value, Wq, Wk, Wv, gating, mask, attn_out, out_t = tensors

    pers = ctx.enter_context(tc.tile_pool(name="pers", bufs=1))

    # --- constants -------------------------------------------------------
    identity = pers.tile([128, 128], F32)
    make_identity(nc, identity)
    identity_r = pers.tile([128, 128], MM_DT)
    nc.vector.tensor_copy(identity_r, identity)
    ones1 = pers.tile([1, 128], F32)
    nc.vector.memset(ones1, 1.0)

    # --- gates: sigmoid via exp + reciprocal (stays in exp table set) ----
    # broadcast gating to all 128 partitions via 0-stride DMA, then compute
    # sigmoid replicated per partition (no PSUM / matmul involved).
    g_in = pers.tile([128, H], F32)
    nc.sync.dma_start(out=g_in, in_=gating.ap().partition_broadcast(128))
    g_eneg = pers.tile([128, H], F32)
    nc.scalar.activation(g_eneg, g_in, AF.Exp, scale=-1.0)
    g_den = pers.tile([128, H], F32)
    nc.vector.tensor_scalar(g_den, g_eneg, 1.0, None, ALU.add)
    gbc = pers.tile([128, 3 * H], F32)  # cols 0..7 = g, 8..15 = 1-g, 16..23 = g/(1-g)
    nc.vector.reciprocal(gbc[:, 0:H], g_den)
    nc.vector.tensor_scalar(gbc[:, H : 2 * H], gbc[:, 0:H], -1.0, 1.0, ALU.mult, ALU.add)
    grat = pers.tile([128, H], F32)
    nc.vector.reciprocal(grat, gbc[:, H : 2 * H])
    nc.vector.tensor_tensor(gbc[:, 2 * H : 3 * H], gbc[:, 0:H], grat, ALU.mult)

    # --- prologue: transposes + projections ------------------------------
    qT_all = pers.tile([128, EB, S], BF16)  # (q W^T)^T: head-dim on partitions
    kT_all = pers.tile([128, EB, S], BF16)
    vp = pers.tile([128, SB, E], MM_DT)      # v natural: [s-block partitions, h*64+d]


    mask_ap = mask.ap()

    with (
        tc.tile_pool(name="pro", bufs=2) as pro,
        tc.tile_pool(name="proT", bufs=1) as proT,
        tc.tile_pool(name="propsum", bufs=2, space="PSUM") as ppsum,
    ):
        # sacrificial transpose: absorbs the gpsimd(identity) wait on PE so
        # later PE instructions don't each need a Pool semaphore wait.
        warm = ppsum.tile([128, S], F32, name="warm", tag="pps")
        nc.tensor.transpose(warm[:, 0:128], identity, identity)

        xT_tiles = {}
        for name, dram in (("query", query), ("key", key), ("value", value)):
            xdt = MM_DT if name == "value" else BF16
            xT = proT.tile([128, EB, S], xdt, name=f"{name}T")
            # one DMA: [s, e] -> [s%128, s//128, e]
            xn = pro.tile([128, SB, E], F32, name=f"{name}_nat", tag="xnat")
            nc.sync.dma_start(
                out=xn, in_=dram.ap().rearrange("(a p) e -> p a e", p=128)
            )
            for eb in range(EB):
                ps = ppsum.tile([128, S], F32, name=f"{name}_tps{eb}", tag="pps")
                for sb in range(SB):
                    nc.tensor.transpose(
                        ps[:, sb * 128 : (sb + 1) * 128],
                        xn[:, sb, eb * 128 : (eb + 1) * 128],
                        identity,
                    )
                if eb % 2 == 0:
                    nc.scalar.copy(xT[:, eb, :], ps)
                else:
                    nc.vector.tensor_copy(xT[:, eb, :], ps)
            xT_tiles[name] = xT

        wT_tiles = {}
        for name, dram in (("Wq", Wq), ("Wk", Wk), ("Wv", Wv)):
            wdt = MM_DT if name == "Wv" else BF16
            wT = proT.tile([128, EB, E], wdt, name=f"{name}T")
            wn = pro.tile([128, EB, E], F32, name=f"{name}_nat", tag="wnat")
            nc.sync.dma_start(
                out=wn, in_=dram.ap().rearrange("(a p) e -> p a e", p=128)
            )
            for eb in range(EB):
                ps = ppsum.tile([128, E], F32, name=f"{name}_tps{eb}", tag="pps")
                for rb in range(EB):
                    nc.tensor.transpose(
                        ps[:, rb * 128 : (rb + 1) * 128],
                        wn[:, rb, eb * 128 : (eb + 1) * 128],
                        identity,
                    )
                if eb % 2 == 0:
                    nc.scalar.copy(wT[:, eb, :], ps)
                else:
                    nc.vector.tensor_copy(wT[:, eb, :], ps)
            wT_tiles[name] = wT

        # q^T/k^T projections: [head-pair p] psum[128, S] = sum_eb WxT[:,eb,p-cols].T @ xT[:,eb,:]
        for dst, wname, xname in ((qT_all, "Wq", "query"), (kT_all, "Wk", "key")):
            wT, xT = wT_tiles[wname], xT_tiles[xname]
            for p in range(EB):
                ps = ppsum.tile([128, S], F32, name=f"{wname}p{p}", tag="pps")
                for eb in range(EB):
                    for nh in range(2):
                        nc.tensor.matmul(
                            ps[:, nh * 512 : (nh + 1) * 512],
                            lhsT=wT[:, eb, p * 128 : (p + 1) * 128],
                            rhs=xT[:, eb, nh * 512 : (nh + 1) * 512],
                            start=(eb == 0),
                            stop=(eb == EB - 1),
                        )
                if p % 2 == 0:
                    nc.scalar.copy(dst[:, p, :], ps)
                else:
                    nc.vector.tensor_copy(dst[:, p, :], ps)

        # v natural: [s-block m] psum[128, E] = sum_eb valueT[:,eb,m-cols].T @ WvT[:,eb,:]
        wvT, valT = wT_tiles["Wv"], xT_tiles["value"]
        for m in range(SB):
            ps = ppsum.tile([128, E], F32, name=f"vp{m}", tag="pps")
            for eb in range(EB):
                nc.tensor.matmul(
                    ps,
                    lhsT=valT[:, eb, m * 128 : (m + 1) * 128],
                    rhs=wvT[:, eb, :],
                    start=(eb == 0),
                    stop=(eb == EB - 1),
                )
            if m % 2 == 0:
                nc.scalar.copy(vp[:, m, :], ps)
            else:
                nc.vector.tensor_copy(vp[:, m, :], ps)

    # --- main loop -------------------------------------------------------
    with (
        tc.tile_pool(name="scps", bufs=2, space="PSUM") as scps_pool,
        tc.tile_pool(name="trps", bufs=1, space="PSUM") as trps_pool,
        tc.tile_pool(name="otps", bufs=1, space="PSUM") as otps_pool,
        tc.tile_pool(name="mask", bufs=3) as mask_pool,
        tc.tile_pool(name="cexp", bufs=4) as cexp_pool,
        tc.tile_pool(name="pexp", bufs=3) as pexp_pool,
        tc.tile_pool(name="t2", bufs=2) as t2_pool,
        tc.tile_pool(name="attn", bufs=3) as attn_pool,
        tc.tile_pool(name="attnT", bufs=2) as attnT_pool,
        tc.tile_pool(name="osb", bufs=2) as osb_pool,
        tc.tile_pool(name="small", bufs=24) as small,
    ):
        pending = None  # deferred out^T accumulation for the previous head
        for h in range(H):
            p, po = h // 2, 64 * (h % 2)
            attnT = attnT_pool.tile([128, SB, S], MM_DT, name=f"attnT{h}", tag="attnT")
            mhalf = {}
            for g2 in range(2):
                mh = mask_pool.tile([128, 4, S], F32, name=f"m{h}_{g2}", tag="mask")
                nc.sync.dma_start(
                    out=mh,
                    in_=mask_ap[h, g2 * 512 : (g2 + 1) * 512, :].rearrange(
                        "(a p) k -> p a k", p=128
                    ),
                )
                mhalf[g2] = mh
            for qb in range(SB):
                qsl = slice(qb * 128, (qb + 1) * 128)
                mt = mhalf[qb // 4][:, qb % 4, :]
                # scores
                sc = scps_pool.tile([128, S], F32, name=f"sc{h}_{qb}", tag="sc")
                for nh in range(2):
                    nc.tensor.matmul(
                        sc[:, nh * 512 : (nh + 1) * 512],
                        lhsT=qT_all[po : po + D, p, qsl],
                        rhs=kT_all[po : po + D, p, nh * 512 : (nh + 1) * 512],
                        start=True,
                        stop=True,
                    )
                # previous head's out^T accumulation, spread 2 matmuls per
                # q-block so the PE never stalls the next head's scores
                if pending is not None:
                    ph = pending["h"]
                    if qb == 0:
                        pending["ot"] = otps_pool.tile(
                            [64, S], F32, name=f"ot{ph}", tag="ot"
                        )
                    for nh in range(2):
                        nc.tensor.matmul(
                            pending["ot"][:, nh * 512 : (nh + 1) * 512],
                            lhsT=vp[:, qb, ph * D : (ph + 1) * D],
                            rhs=pending["attnT"][:, qb, nh * 512 : (nh + 1) * 512],
                            start=(qb == 0),
                            stop=(qb == SB - 1),
                        )
                    if qb == SB - 1:
                        osb = osb_pool.tile([64, S], F32, name=f"ot_sb{ph}", tag="osb")
                        nc.scalar.copy(osb, pending["ot"])
                        nc.sync.dma_start(out=out_t.ap()[ph], in_=osb)
                # exp + row sums
                cexp = cexp_pool.tile([128, S], F32, name=f"ce{h}_{qb}", tag="cexp")
                sums = small.tile([128, 2], F32, name=f"su{h}_{qb}", tag="sums")
                nc.scalar.activation(cexp, sc, AF.Exp, scale=SCALE, accum_out=sums[:, 0:1])
                pexp = pexp_pool.tile([128, S], F32, name=f"pe{h}_{qb}", tag="pexp")
                nc.scalar.activation(pexp, mt, AF.Exp, accum_out=sums[:, 1:2])
                # per-row gate coefficients
                rec = small.tile([128, 2], F32, name=f"re{h}_{qb}", tag="rec")
                nc.vector.reciprocal(rec, sums)
                arow = small.tile([128, 1], F32, name=f"ar{h}_{qb}", tag="ar")
                nc.vector.tensor_scalar(arow, rec[:, 0:1], gbc[:, H + h : H + h + 1], None, ALU.mult)
                brow = small.tile([128, 1], F32, name=f"br{h}_{qb}", tag="br")
                nc.vector.tensor_scalar(brow, rec[:, 1:2], gbc[:, h : h + 1], None, ALU.mult)
                # attn = cexp*arow + pexp*brow
                t2 = t2_pool.tile([128, S], F32, name=f"t2{h}_{qb}", tag="t2")
                nc.vector.tensor_scalar(t2, pexp, brow, None, ALU.mult)
                attn_sb = attn_pool.tile([128, S], MM_DT, name=f"at{h}_{qb}", tag="attn")
                nc.vector.scalar_tensor_tensor(
                    attn_sb, cexp, arow, t2, ALU.mult, ALU.add
                )
                nc.sync.dma_start(out=attn_out.ap()[h, qsl, :], in_=attn_sb.bitcast(F32))
                # transpose attn -> attn_T
                tps = trps_pool.tile([128, S], MM_DT, name=f"tp{h}_{qb}", tag="tps")
                for kb in range(SB):
                    nc.tensor.transpose(
                        tps[:, kb * 128 : (kb + 1) * 128],
                        attn_sb[:, kb * 128 : (kb + 1) * 128],
                        identity_r,
                    )
                src3 = tps.rearrange("p (k q) -> p k q", k=SB)
                dst3 = attnT[:, :, qsl]
                if qb % 2 == 0:
                    nc.scalar.copy(dst3, src3)
                else:
                    nc.vector.tensor_copy(dst3, src3)
            pending = {"h": h, "attnT": attnT}
        # flush the final head's out^T
        ph = pending["h"]
        ot = otps_pool.tile([64, S], F32, name=f"ot{ph}", tag="ot")
        for kb in range(SB):
            for nh in range(2):
                nc.tensor.matmul(
                    ot[:, nh * 512 : (nh + 1) * 512],
                    lhsT=vp[:, kb, ph * D : (ph + 1) * D],
                    rhs=pending["attnT"][:, kb, nh * 512 : (nh + 1) * 512],
                    start=(kb == 0),
                    stop=(kb == SB - 1),
                )
        osb = osb_pool.tile([64, S], F32, name=f"ot_sb{ph}", tag="osb")
        nc.scalar.copy(osb, ot)
        nc.sync.dma_start(out=out_t.ap()[ph], in_=osb)


def _build():
    nc = bacc.Bacc("TRN2", target_bir_lowering=False, debug=False)
    query = nc.dram_tensor("query", [S, E], F32, kind="ExternalInput")
    key = nc.dram_tensor("key", [S, E], F32, kind="ExternalInput")
    value = nc.dram_tensor("value", [S, E], F32, kind="ExternalInput")
    Wq = nc.dram_tensor("Wq", [E, E], F32, kind="ExternalInput")
    Wk = nc.dram_tensor("Wk", [E, E], F32, kind="ExternalInput")
    Wv = nc.dram_tensor("Wv", [E, E], F32, kind="ExternalInput")
    gating = nc.dram_tensor("gating", [H], F32, kind="ExternalInput")
    mask = nc.dram_tensor("mask", [H, S, S], F32, kind="ExternalInput")
    attn_out = nc.dram_tensor("attn_out", [H, S, S], F32, kind="ExternalOutput")
    out_t = nc.dram_tensor("out_t", [H, D, S], F32, kind="ExternalOutput")

    from contextlib import ExitStack

    with tile.TileContext(nc) as tc, ExitStack() as ctx:
        _emit(
            nc, tc, ctx,
            (query, key, value, Wq, Wk, Wv, gating, mask, attn_out, out_t),
        )
    nc.compile()
    return nc


def kernel(query, key, value, attn_mask, Wq, Wk, Wv, gating_param):
    global LAST_RESULTS
    query = np.ascontiguousarray(np.asarray(query, dtype=np.float32))
    key = np.ascontiguousarray(np.asarray(key, dtype=np.float32))
    value = np.ascontiguousarray(np.asarray(value, dtype=np.float32))
    attn_mask = np.ascontiguousarray(np.asarray(attn_mask, dtype=np.float32))
    Wq = np.ascontiguousarray(np.asarray(Wq, dtype=np.float32))
    Wk = np.ascontiguousarray(np.asarray(Wk, dtype=np.float32))
    Wv = np.ascontiguousarray(np.asarray(Wv, dtype=np.float32))
    gating_param = np.ascontiguousarray(np.asarray(gating_param, dtype=np.float32))
    B = query.shape[0]

    nc = _build()

    mask4 = attn_mask.reshape(B, H, S, S)
    in_maps = []
    for c in range(B):
        in_maps.append(
            {
                "query": query[c],
                "key": key[c],
                "value": value[c],
                "Wq": Wq,
                "Wk": Wk,
                "Wv": Wv,
                "gating": gating_param,
                "mask": mask4[c],
            }
        )

    trace = bool(int(os.environ.get("KERNEL_TRACE", "0")))
    res = run_bass_kernel_spmd(
        nc, in_maps, core_ids=list(range(NCORES)), trace=trace
    )
    LAST_RESULTS = res

    attn = np.stack([res.results[c]["attn_out"] for c in range(B)], axis=0)
    out_t = np.stack([res.results[c]["out_t"] for c in range(B)], axis=0)
    # out[b, s, h*64+d] = out_t[b, h, d, s]
    out = out_t.transpose(0, 3, 1, 2).reshape(B, S, E)
    return out, attn
